# revision 1
# baseline (speedup 1.0000x reference)
"""Cross-attention block (nn_CABlock) on 8 TRN2 NeuronCores.

Reference (per batch b):
    q  = xq @ Wq.T            -> [SQ, H]   split heads [SQ, 16, 64]
    kv = xkv @ Wkv.T          -> [SKV, 2H] split [SKV, 2, 16, 64]
    att = softmax(q k^T / sqrt(64))
    x   = att @ v  (merge heads)
    out = x @ Wout.T + bout

Sharding: 8 cores = 4 batches x 2 head-groups (8 heads each).  Each core
computes its batch's projections restricted to its 8 heads, attention for
those heads, and a partial out-projection (contraction over its 512
hd-columns of Wout).  Host sums the two partials per batch and adds bout.

All matmuls run in fp32r (fp32 storage, reduced-precision PE mode at bf16
speed, ~1.5e-4 matmul rel err).  Per-core layout:
  - Host pre-transposes activations: xqT [H, SQ], xkvT [H, SKV].
  - q/k projections produce qT/kT with hd on partitions in natural head
    order (head = hd//64, so head h lives in chunk h//2 at partition
    offset (h%2)*64 -- all partition offsets stay 32-aligned).
  - v projection produces v_aug [SKV, 8 heads x 65] with a ones column
    per head: att @ v_aug then also emits the softmax denominator Z as
    row 64.  No max subtraction (scores are O(1), exp cannot overflow).
  - scores are computed transposed, sT[y, x], two 128-y tiles into one
    2-bank PSUM tile; a single W=1024 Exp on the scalar engine (fuses the
    1/8 scale) amortizes the ~250ns ACT fixed cost; fp32r attention
    weights feed att @ v_aug accumulation into psum [65, 512].
  - xTu [512, SQ] collects unnormalized head outputs (natural hd order);
    Z rows are replicated to 128 partitions with K=1 ones-matmuls,
    reciprocal'd on DVE, and multiplied in place once a (2j, 2j+1) head
    pair completes.
  - out-projection contracts the 4 xTu chunks with WoutT slices (host
    pre-transposed, natural order), streaming [128, 512] tiles to DRAM.
"""

import sys

sys.path.insert(0, "/opt/trn_rl_repo")

import numpy as np

import concourse.bass as bass
import concourse.mybir as mybir
import concourse.tile as tile
from concourse.bass_utils import run_bass_kernel_spmd
from concourse.tile import add_dep_helper

F32 = mybir.dt.float32
F32R = mybir.dt.float32r

HIDDEN = 1024
NUM_HEADS = 16
HEAD_DIM = 64
B = 4
SQ = 1024
SKV = 2048
NCORES = 8
NHL = 8          # heads per core
HL = NHL * HEAD_DIM  # 512, local hd width
SCALE = HEAD_DIM ** -0.5
KCH = HIDDEN // 128  # 8 contraction chunks for the projections
NYC = SKV // 128     # 16 key/value row chunks


def _legalize_waits(nc, limit=1):
    """The walrus build in this container accepts only ~1 sync-wait per
    instruction struct; spill excess waits onto preceding engine NoOps."""
    import bass_rust

    ctr = 0
    for fn in nc.m.functions:
        for blk in fn.blocks:
            out = []
            changed = False
            for inst in blk.instructions:
                si = inst.sync_info
                ws = list(si.on_wait) if si is not None and si.on_wait else []
                if len(ws) > limit:
                    spill, keep = ws[:-limit], ws[-limit:]
                    for w in spill:
                        ctr += 1
                        nop = mybir.InstNoOp(name=f"ant-waitnop-{ctr}", ins=[], outs=[])
                        nop.engine = inst.engine
                        nop.sync_info = bass_rust.SyncInfo(on_wait=[w], on_update=[])
                        out.append(nop)
                    si.on_wait = keep
                    changed = True
                out.append(inst)
            if changed:
                blk.instructions = out
    return ctr


def _emit(nc, tc):
    xqT = nc.dram_tensor("xqT", [HIDDEN, SQ], F32R, kind="ExternalInput")
    xkvT = nc.dram_tensor("xkvT", [HIDDEN, SKV], F32R, kind="ExternalInput")
    WqT = nc.dram_tensor("WqT", [HIDDEN, HL], F32R, kind="ExternalInput")
    WkT = nc.dram_tensor("WkT", [HIDDEN, HL], F32R, kind="ExternalInput")
    WvT = nc.dram_tensor("WvT", [HIDDEN, HL], F32R, kind="ExternalInput")
    WoT = nc.dram_tensor("WoT", [HL, HIDDEN], F32R, kind="ExternalInput")
    onesA = nc.dram_tensor("onesA", [1, 128], F32R, kind="ExternalInput")
    onesB = nc.dram_tensor("onesB", [1, 128], F32R, kind="ExternalInput")
    out_d = nc.dram_tensor("out", [SQ, HIDDEN], F32, kind="ExternalOutput")

    with tc.tile_pool(name="persist", bufs=1) as pp:
        qT = pp.tile([128, 4, SQ], F32R)          # 16 KB/part
        onesA_t = pp.tile([1, 128], F32R)
        onesB_t = pp.tile([1, 128], F32R)
        kT = pp.tile([128, 4, SKV], F32R)         # 32 KB
        va = pp.tile([128, NYC, NHL, 65], F32R)   # 33.3 KB  (y-chunk, head, dv+1)
        xTu = pp.tile([128, 4, SQ], F32R)         # 16 KB, natural hd order
        nc.sync.dma_start(out=onesA_t, in_=onesA[:, :])
        nc.sync.dma_start(out=onesB_t, in_=onesB[:, :])

        with tc.tile_pool(name="psProj", bufs=4, space="PSUM") as psProj:
            # ---- Phase 1: q projection  qT[hd, s] = sum_i Wq[hd, i] xq[s, i]
            with tc.tile_pool(name="qin", bufs=1) as qin:
                xqT_t = qin.tile([128, KCH, SQ], F32R)
                WqT_t = qin.tile([128, KCH, HL], F32R)
                nc.sync.dma_start(
                    out=WqT_t, in_=WqT.rearrange("(k p) n -> p k n", p=128)
                )
                for k in range(KCH):
                    nc.sync.dma_start(
                        out=xqT_t[:, k, :], in_=xqT[k * 128 : (k + 1) * 128, :]
                    )
                # k outermost: compute starts as soon as the first chunks land
                for s in range(2):
                    pts = [psProj.tile([128, 512], F32, tag="proj", name="ptq") for _ in range(4)]
                    for k in range(KCH):
                        for m in range(4):
                            nc.tensor.matmul(
                                pts[m][:, :],
                                lhsT=WqT_t[:, k, m * 128 : (m + 1) * 128],
                                rhs=xqT_t[:, k, s * 512 : (s + 1) * 512],
                                start=(k == 0),
                                stop=(k == KCH - 1),
                            )
                    for m in range(4):
                        nc.vector.tensor_copy(
                            out=qT[:, m, s * 512 : (s + 1) * 512], in_=pts[m][:, :]
                        )

            # ---- Phase 2: k and v projections from xkvT
            with tc.tile_pool(name="kvx", bufs=1) as kvx:
                xkvT_t = kvx.tile([128, KCH, SKV], F32R)
                for k in range(KCH):
                    nc.sync.dma_start(
                        out=xkvT_t[:, k, :], in_=xkvT[k * 128 : (k + 1) * 128, :]
                    )
                with tc.tile_pool(name="kvw1", bufs=1) as kvw1:
                    WkT_t = kvw1.tile([128, KCH, HL], F32R)
                    nc.sync.dma_start(
                        out=WkT_t, in_=WkT.rearrange("(k p) n -> p k n", p=128)
                    )
                    # kT[hd, y], k outermost within each y-group
                    for yg in range(SKV // 512):
                        pts = [
                            psProj.tile([128, 512], F32, tag="proj", name="ptk")
                            for _ in range(4)
                        ]
                        for k in range(KCH):
                            for m in range(4):
                                nc.tensor.matmul(
                                    pts[m][:, :],
                                    lhsT=WkT_t[:, k, m * 128 : (m + 1) * 128],
                                    rhs=xkvT_t[:, k, yg * 512 : (yg + 1) * 512],
                                    start=(k == 0),
                                    stop=(k == KCH - 1),
                                )
                        for m in range(4):
                            nc.vector.tensor_copy(
                                out=kT[:, m, yg * 512 : (yg + 1) * 512],
                                in_=pts[m][:, :],
                            )
                with tc.tile_pool(name="kvw2", bufs=1) as kvw2:
                    WvT_t = kvw2.tile([128, KCH, HL], F32R)
                    nc.sync.dma_start(
                        out=WvT_t, in_=WvT.rearrange("(k p) n -> p k n", p=128)
                    )
                    # v_aug[y, h, 0:64] in two half-width passes (heads 0-3,
                    # then 4-7) so early attention chains unblock sooner
                    for half in range(2):
                        hlo = half * 4
                        for yc in range(NYC):
                            pt = psProj.tile([128, 512], F32, tag="proj")
                            for k in range(KCH):
                                nc.tensor.matmul(
                                    pt[:, 0:256],
                                    lhsT=xkvT_t[:, k, yc * 128 : (yc + 1) * 128],
                                    rhs=WvT_t[:, k, hlo * 64 : (hlo + 4) * 64],
                                    start=(k == 0),
                                    stop=(k == KCH - 1),
                                )
                            nc.vector.tensor_copy(
                                out=va[:, yc, hlo : hlo + 4, 0:64],
                                in_=pt[:, 0:256].rearrange("p (h d) -> p h d", h=4),
                            )
                    nc.vector.memset(va[:, :, :, 64:65].bitcast(F32), 1.0)

        # ---- Phase 3: attention + normalization + out projection
        with (
            tc.tile_pool(name="attp", bufs=1) as attp,
            tc.tile_pool(name="zrowp", bufs=4) as zrowp,
            tc.tile_pool(name="rzp", bufs=2) as rzp,
            tc.tile_pool(name="outw", bufs=1) as outw,
            tc.tile_pool(name="outstage", bufs=3) as outstage,
            tc.tile_pool(name="psS2", bufs=1, space="PSUM") as psS2,
            tc.tile_pool(name="psMisc", bufs=1, space="PSUM") as psMisc,
        ):
            WoT_t = outw.tile([128, 4, HIDDEN], F32R)
            nc.sync.dma_start(
                out=WoT_t, in_=WoT.rearrange("(j p) n -> p j n", p=128)
            )

            NYB = NYC // 2  # 8 double-y blocks per chain
            LAG = 3         # attv trails scores/exp by this many blocks

            _scnt = [0]

            def emit_scores(h, xc, yb):
                pq = (h % 2) * 64
                m = h // 2
                _scnt[0] += 1
                pscr = psS2.tile(
                    [128, 2, 512], F32, tag=f"scores{_scnt[0] % 3}", name="pscr"
                )
                last = None
                for i in range(2):
                    yc = 2 * yb + i
                    last = nc.tensor.matmul(
                        pscr[:, i, :],
                        lhsT=kT[pq : pq + 64, m, yc * 128 : (yc + 1) * 128],
                        rhs=qT[pq : pq + 64, m, xc * 512 : (xc + 1) * 512],
                        start=True,
                        stop=True,
                    )
                at = attp.tile(
                    [128, 2, 512], F32R, tag=f"att{_scnt[0] % 10}", name="at"
                )
                nc.scalar.activation(
                    out=at[:, :, :].rearrange("p a b -> p (a b)"),
                    in_=pscr[:, :, :].rearrange("p a b -> p (a b)"),
                    func=mybir.ActivationFunctionType.Exp,
                    scale=SCALE,
                )
                return at, last

            def emit_attv(h, po, at, yb, order_after=None):
                for i in range(2):
                    yc = 2 * yb + i
                    mm = nc.tensor.matmul(
                        po[:, :],
                        lhsT=va[:, yc, h, :],
                        rhs=at[:, i, :],
                        start=(yc == 0),
                        stop=(yc == NYC - 1),
                    )
                    if i == 0 and order_after is not None:
                        # force the PE static order to keep attv trailing the
                        # scores stream by LAG blocks (hides the ACT latency)
                        add_dep_helper(
                            mm.ins,
                            order_after.ins,
                            sync=False,
                            reason="attv trails scores pipeline",
                        )

            for xc in range(SQ // 512):
                for j in range(4):
                    hA, hB = 2 * j, 2 * j + 1
                    poA = psMisc.tile([65, 512], F32, tag="attv", bufs=1, name="poA")
                    poB = psMisc.tile([65, 512], F32, tag="attv2", bufs=1, name="poB")
                    attsA, attsB = [], []
                    # two independent chains interleaved: while one waits on
                    # ACT, the PE works the other
                    scoreMM = []
                    for yb in range(NYB):
                        atA, _ = emit_scores(hA, xc, yb)
                        atB, lastB = emit_scores(hB, xc, yb)
                        attsA.append(atA)
                        attsB.append(atB)
                        scoreMM.append(lastB)
                        if yb >= LAG:
                            emit_attv(hA, poA, attsA[yb - LAG], yb - LAG,
                                      order_after=scoreMM[yb])
                            emit_attv(hB, poB, attsB[yb - LAG], yb - LAG)
                    for yb in range(NYB - LAG, NYB):
                        emit_attv(hA, poA, attsA[yb], yb)
                        emit_attv(hB, poB, attsB[yb], yb)

                    zrs = []
                    for h, po in ((hA, poA), (hB, poB)):
                        ps_off = (h % 2) * 64
                        nc.vector.tensor_copy(
                            out=xTu[
                                ps_off : ps_off + 64, j, xc * 512 : (xc + 1) * 512
                            ],
                            in_=po[0:64, :],
                        )
                        zr = zrowp.tile([1, 512], F32R, tag="zrow", name="zr")
                        nc.vector.tensor_copy(out=zr[0:1, :], in_=po[64:65, :])
                        zrs.append(zr)
                    # normalize chunk j (bcast psum shares the scores slots)
                    pb = psS2.tile([128, 512], F32, tag="scores0", name="pb")
                    nc.tensor.matmul(
                        pb[:, :], lhsT=onesA_t[0:1, :], rhs=zrs[0][0:1, :],
                        start=True, stop=False,
                    )
                    nc.tensor.matmul(
                        pb[:, :], lhsT=onesB_t[0:1, :], rhs=zrs[1][0:1, :],
                        start=False, stop=True,
                    )
                    rz = rzp.tile([128, 512], F32, tag="rz", name="rz")
                    nc.vector.reciprocal(out=rz[:, :], in_=pb[:, :])
                    nc.vector.tensor_mul(
                        xTu[:, j, xc * 512 : (xc + 1) * 512],
                        xTu[:, j, xc * 512 : (xc + 1) * 512],
                        rz[:, :],
                    )

            # out projection: out[s, o] = sum_j xTu[:, j, s].T @ WoT[:, j, o]
            for sc in range(SQ // 128):
                for oc in range(HIDDEN // 512):
                    pt = psS2.tile([128, 512], F32, tag="scores1", name="pto")
                    for j in range(4):
                        nc.tensor.matmul(
                            pt[:, :],
                            lhsT=xTu[:, j, sc * 128 : (sc + 1) * 128],
                            rhs=WoT_t[:, j, oc * 512 : (oc + 1) * 512],
                            start=(j == 0),
                            stop=(j == 3),
                        )
                    ot = outstage.tile([128, 512], F32, tag="out")
                    nc.vector.tensor_copy(out=ot[:, :], in_=pt[:, :])
                    nc.sync.dma_start(
                        out=out_d[sc * 128 : (sc + 1) * 128, oc * 512 : (oc + 1) * 512],
                        in_=ot[:, :],
                    )


_NC = None


def _get_nc():
    global _NC
    if _NC is None:
        nc = bass.Bass(trn_type="TRN2")
        with tile.TileContext(nc) as tc:
            _emit(nc, tc)
        _legalize_waits(nc)
        _NC = nc
    return _NC


def _prep_inputs(xq, xkv, Wq, Wkv, Wout):
    xq = np.asarray(xq, dtype=np.float32)
    xkv = np.asarray(xkv, dtype=np.float32)
    Wq = np.asarray(Wq, dtype=np.float32)
    Wkv = np.asarray(Wkv, dtype=np.float32)
    Wout = np.asarray(Wout, dtype=np.float32)

    onesA = np.zeros((1, 128), np.float32)
    onesA[0, 0:64] = 1.0
    onesB = np.zeros((1, 128), np.float32)
    onesB[0, 64:128] = 1.0

    xqT = [np.ascontiguousarray(xq[b].T) for b in range(B)]
    xkvT = [np.ascontiguousarray(xkv[b].T) for b in range(B)]

    per_hg = []
    for hg in range(2):
        hs = slice(hg * HL, (hg + 1) * HL)
        WqTh = np.ascontiguousarray(Wq[hs].T)
        WkTh = np.ascontiguousarray(Wkv[hs].T)
        WvTh = np.ascontiguousarray(Wkv[HIDDEN + hg * HL : HIDDEN + (hg + 1) * HL].T)
        WoTh = np.ascontiguousarray(Wout[:, hs].T)
        per_hg.append((WqTh, WkTh, WvTh, WoTh))

    in_maps = []
    for c in range(NCORES):
        b, hg = c // 2, c % 2
        WqTh, WkTh, WvTh, WoTh = per_hg[hg]
        in_maps.append(
            {
                "xqT": xqT[b],
                "xkvT": xkvT[b],
                "WqT": WqTh,
                "WkT": WkTh,
                "WvT": WvTh,
                "WoT": WoTh,
                "onesA": onesA,
                "onesB": onesB,
            }
        )
    return in_maps


def run_sharded(xq, xkv, Wq, Wkv, Wout, bout, trace=False, **kwargs):
    """Build+run the SPMD kernel; returns (full_output, BassKernelResults)."""
    nc = _get_nc()
    in_maps = _prep_inputs(xq, xkv, Wq, Wkv, Wout)
    res = run_bass_kernel_spmd(
        nc, in_maps, core_ids=list(range(NCORES)), trace=trace, **kwargs
    )
    bout = np.asarray(bout, dtype=np.float32)
    out = np.empty((B, SQ, HIDDEN), np.float32)
    for b in range(B):
        out[b] = res.results[2 * b]["out"] + res.results[2 * b + 1]["out"]
    out += bout[None, None, :]
    return out, res


def kernel(xq, xkv, Wq, Wkv, Wout, bout):
    out, _ = run_sharded(xq, xkv, Wq, Wkv, Wout, bout)
    return out



# revision 8
# speedup vs baseline: 1.2594x; 1.2594x over previous
"""Cross-attention block (nn_CABlock) on 8 TRN2 NeuronCores.

Reference (per batch b):
    q  = xq @ Wq.T            -> [SQ, H]   split heads [SQ, 16, 64]
    kv = xkv @ Wkv.T          -> [SKV, 2H] split [SKV, 2, 16, 64]
    att = softmax(q k^T / sqrt(64))
    x   = att @ v  (merge heads)
    out = x @ Wout.T + bout

Sharding: 8 cores = 4 batches x 2 head-groups (8 heads each).  Each core
computes its batch's projections restricted to its 8 heads, attention for
those heads, and a partial out-projection (contraction over its 512
hd-columns of Wout).  Host sums the two partials per batch and adds bout.

v2 design (fused pipeline, all bf16):
  - Everything is bf16 on the wire and in SBUF (halves DMA + SBUF, and
    bf16 moving operands run 1 cycle/row on the PE at any N).  PSUM
    accumulation stays fp32, final partials are summed on the host in
    fp32.
  - No phases: q/k/v projection units, attention chains (scores -> exp
    -> att@v), and out-projection units are emitted interleaved with
    minimal dependencies.  The Tile list-scheduler fills PE idle slots
    (waiting on the ACT-bound exp stream) with ready projection
    matmuls, keeping the PE continuously busy (max p-state).
  - PSUM budget (8 banks): 2 rotating score slots [128,2,512] (4
    banks), 2 att@v accumulators [65,512] (2 banks), 2 shared slots
    [128,512] for proj/bcast/out-proj (2 banks).
  - softmax denominator comes from a ones-column in the v tile (att @
    v_aug emits Z as row 64); normalization uses a ones-matmul
    partition broadcast + reciprocal_approx_fast (5x faster than
    reciprocal) + one in-place multiply.
  - Input DMAs are dispatched from two engines (sync: q-side, scalar:
    kv-side) and chunked so projection compute starts on first-chunk
    arrival.
"""

import sys

sys.path.insert(0, "/opt/trn_rl_repo")

import numpy as np

import concourse.bass as bass
import concourse.mybir as mybir
import concourse.tile as tile
from concourse.bass_utils import run_bass_kernel_spmd

F32 = mybir.dt.float32
F32R = mybir.dt.float32r
BF16 = mybir.dt.bfloat16

HIDDEN = 1024
NUM_HEADS = 16
HEAD_DIM = 64
B = 4
SQ = 1024
SKV = 2048
NCORES = 8
NHL = 8          # heads per core
HL = NHL * HEAD_DIM  # 512, local hd width
SCALE = HEAD_DIM ** -0.5
KCH = HIDDEN // 128  # 8 contraction chunks for the projections
NYC = SKV // 128     # 16 key/value row chunks


def _legalize_waits(nc, limit=1):
    """The walrus build in this container accepts only ~1 sync-wait per
    instruction struct; spill excess waits onto preceding engine NoOps."""
    import bass_rust

    ctr = 0
    for fn in nc.m.functions:
        for blk in fn.blocks:
            out = []
            changed = False
            for inst in blk.instructions:
                si = inst.sync_info
                ws = list(si.on_wait) if si is not None and si.on_wait else []
                if len(ws) > limit:
                    spill, keep = ws[:-limit], ws[-limit:]
                    for w in spill:
                        ctr += 1
                        nop = mybir.InstNoOp(name=f"ant-waitnop-{ctr}", ins=[], outs=[])
                        nop.engine = inst.engine
                        nop.sync_info = bass_rust.SyncInfo(on_wait=[w], on_update=[])
                        out.append(nop)
                    si.on_wait = keep
                    changed = True
                out.append(inst)
            if changed:
                blk.instructions = out
    return ctr


def _emit(nc, tc):
    xqT = nc.dram_tensor("xqT", [HIDDEN, SQ], BF16, kind="ExternalInput")
    xkvT = nc.dram_tensor("xkvT", [HIDDEN, SKV], BF16, kind="ExternalInput")
    WqT = nc.dram_tensor("WqT", [HIDDEN, HL], BF16, kind="ExternalInput")
    WkT = nc.dram_tensor("WkT", [HIDDEN, HL], BF16, kind="ExternalInput")
    WvT = nc.dram_tensor("WvT", [HIDDEN, HL], BF16, kind="ExternalInput")
    WoT = nc.dram_tensor("WoT", [HL, HIDDEN], BF16, kind="ExternalInput")
    onesA = nc.dram_tensor("onesA", [1, 128], BF16, kind="ExternalInput")
    onesB = nc.dram_tensor("onesB", [1, 128], BF16, kind="ExternalInput")
    out_d = nc.dram_tensor("out", [SQ, HIDDEN], BF16, kind="ExternalOutput")

    with tc.tile_pool(name="persist", bufs=1) as pp:
        # persistent SBUF (bytes/partition):
        xqT_t = pp.tile([128, KCH, SQ], BF16)      # 16 KB
        xkvT_t = pp.tile([128, KCH, SKV], BF16)    # 32 KB
        WqT_t = pp.tile([128, KCH, HL], BF16)      # 8 KB
        WkT_t = pp.tile([128, KCH, HL], BF16)      # 8 KB
        WvT_t = pp.tile([128, KCH, HL], BF16)      # 8 KB
        WoT_t = pp.tile([128, 4, HIDDEN], BF16)    # 8 KB
        qT = pp.tile([128, 4, SQ], BF16)           # 8 KB
        kT = pp.tile([128, 4, SKV], BF16)          # 16 KB
        va = pp.tile([128, NYC, NHL, 65], BF16)    # 16.3 KB
        xTu = pp.tile([128, 4, SQ], BF16)          # 8 KB
        onesA_t = pp.tile([1, 128], BF16)
        onesB_t = pp.tile([1, 128], BF16)

        # ---- input DMAs: q-side stream on sync, kv-side stream on scalar
        nc.sync.dma_start(out=onesA_t, in_=onesA[:, :])
        nc.sync.dma_start(out=onesB_t, in_=onesB[:, :])
        nc.sync.dma_start(out=WqT_t, in_=WqT.rearrange("(k p) n -> p k n", p=128))
        for k in range(KCH):
            nc.sync.dma_start(out=xqT_t[:, k, :], in_=xqT[k * 128 : (k + 1) * 128, :])
        nc.scalar.dma_start(out=WkT_t, in_=WkT.rearrange("(k p) n -> p k n", p=128))
        for k in range(KCH):
            nc.scalar.dma_start(
                out=xkvT_t[:, k, :], in_=xkvT[k * 128 : (k + 1) * 128, :]
            )
        nc.scalar.dma_start(out=WvT_t, in_=WvT.rearrange("(k p) n -> p k n", p=128))
        nc.scalar.dma_start(out=WoT_t, in_=WoT.rearrange("(j p) n -> p j n", p=128))

        with (
            tc.tile_pool(name="psS", bufs=1, space="PSUM") as psS,
            tc.tile_pool(name="psPO", bufs=1, space="PSUM") as psPO,
            tc.tile_pool(name="psP", bufs=1, space="PSUM") as psP,
            tc.tile_pool(name="attp", bufs=1) as attp,
            tc.tile_pool(name="zrp", bufs=4) as zrp,
            tc.tile_pool(name="rzp", bufs=2) as rzp,
            tc.tile_pool(name="outst", bufs=3) as outst,
        ):
            nc.vector.memset(va[:, :, :, 64:65], 1.0)

            _pcnt = [0]

            def p_slot():
                _pcnt[0] += 1
                return psP.tile([128, 512], F32, tag=f"P{_pcnt[0] % 2}", name="pslot")

            def emit_qu(m):
                # q projection for head pair m, both x windows
                for s in range(2):
                    pt = p_slot()
                    for k in range(KCH):
                        nc.tensor.matmul(
                            pt[:, :],
                            lhsT=WqT_t[:, k, m * 128 : (m + 1) * 128],
                            rhs=xqT_t[:, k, s * 512 : (s + 1) * 512],
                            start=(k == 0),
                            stop=(k == KCH - 1),
                        )
                    nc.vector.tensor_copy(
                        out=qT[:, m, s * 512 : (s + 1) * 512], in_=pt[:, :]
                    )

            def emit_ku(m):
                # k projection for head pair m, all y
                for yg in range(SKV // 512):
                    pt = p_slot()
                    for k in range(KCH):
                        nc.tensor.matmul(
                            pt[:, :],
                            lhsT=WkT_t[:, k, m * 128 : (m + 1) * 128],
                            rhs=xkvT_t[:, k, yg * 512 : (yg + 1) * 512],
                            start=(k == 0),
                            stop=(k == KCH - 1),
                        )
                    nc.vector.tensor_copy(
                        out=kT[:, m, yg * 512 : (yg + 1) * 512], in_=pt[:, :]
                    )

            def emit_vu(half):
                # v projection for heads [4*half, 4*half+4), all y chunks
                hlo = half * 4
                for yc in range(NYC):
                    pt = p_slot()
                    for k in range(KCH):
                        nc.tensor.matmul(
                            pt[:, 0:256],
                            lhsT=xkvT_t[:, k, yc * 128 : (yc + 1) * 128],
                            rhs=WvT_t[:, k, hlo * 64 : (hlo + 4) * 64],
                            start=(k == 0),
                            stop=(k == KCH - 1),
                        )
                    nc.vector.tensor_copy(
                        out=va[:, yc, hlo : hlo + 4, 0:64],
                        in_=pt[:, 0:256].rearrange("p (h d) -> p h d", h=4),
                    )

            NYB = NYC // 2  # 8 double-y blocks per chain
            LAG = 3         # attv trails scores/exp by this many blocks
            _scnt = [0]
            _acnt = [0]

            def chain(xc, j):
                hA, hB = 2 * j, 2 * j + 1
                poA = psPO.tile([65, 512], F32, tag="POA", name="poA")
                poB = psPO.tile([65, 512], F32, tag="POB", name="poB")
                attsA, attsB = [], []

                def scores(h, yb):
                    pq = (h % 2) * 64
                    m = h // 2
                    _scnt[0] += 1
                    pscr = psS.tile(
                        [128, 2, 512], F32, tag=f"S{_scnt[0] % 2}", name="pscr"
                    )
                    for i in range(2):
                        yc = 2 * yb + i
                        nc.tensor.matmul(
                            pscr[:, i, :],
                            lhsT=kT[pq : pq + 64, m, yc * 128 : (yc + 1) * 128],
                            rhs=qT[pq : pq + 64, m, xc * 512 : (xc + 1) * 512],
                            start=True,
                            stop=True,
                        )
                    _acnt[0] += 1
                    at = attp.tile(
                        [128, 2, 512], BF16, tag=f"att{_acnt[0] % 10}", name="at"
                    )
                    nc.scalar.activation(
                        out=at[:, :, :].rearrange("p a b -> p (a b)"),
                        in_=pscr[:, :, :].rearrange("p a b -> p (a b)"),
                        func=mybir.ActivationFunctionType.Exp,
                        scale=SCALE,
                    )
                    return at

                def attv(h, po, at, yb):
                    for i in range(2):
                        yc = 2 * yb + i
                        nc.tensor.matmul(
                            po[:, :],
                            lhsT=va[:, yc, h, :],
                            rhs=at[:, i, :],
                            start=(yc == 0),
                            stop=(yc == NYC - 1),
                        )

                for yb in range(NYB):
                    attsA.append(scores(hA, yb))
                    attsB.append(scores(hB, yb))
                    if yb >= LAG:
                        attv(hA, poA, attsA[yb - LAG], yb - LAG)
                        attv(hB, poB, attsB[yb - LAG], yb - LAG)
                for yb in range(NYB - LAG, NYB):
                    attv(hA, poA, attsA[yb], yb)
                    attv(hB, poB, attsB[yb], yb)

                zrs = []
                for hi, po in ((0, poA), (1, poB)):
                    ps_off = hi * 64
                    nc.vector.tensor_copy(
                        out=xTu[ps_off : ps_off + 64, j, xc * 512 : (xc + 1) * 512],
                        in_=po[0:64, :],
                    )
                    zr = zrp.tile([1, 512], BF16, tag=f"zr{hi}", name="zr")
                    nc.vector.tensor_copy(out=zr[0:1, :], in_=po[64:65, :])
                    zrs.append(zr)
                # broadcast Z of both heads across partitions, reciprocal,
                # then normalize xTu in place
                pb = p_slot()
                nc.tensor.matmul(
                    pb[:, :], lhsT=onesA_t[0:1, :], rhs=zrs[0][0:1, :],
                    start=True, stop=False,
                )
                nc.tensor.matmul(
                    pb[:, :], lhsT=onesB_t[0:1, :], rhs=zrs[1][0:1, :],
                    start=False, stop=True,
                )
                rz = rzp.tile([128, 512], F32, tag="rz", name="rz")
                nc.vector.reciprocal(out=rz[:, :], in_=pb[:, :])
                nc.vector.tensor_mul(
                    xTu[:, j, xc * 512 : (xc + 1) * 512],
                    xTu[:, j, xc * 512 : (xc + 1) * 512],
                    rz[:, :],
                )

            def emit_ou(sc):
                # out projection for s rows [sc*128, (sc+1)*128)
                for oc in range(HIDDEN // 512):
                    pt = p_slot()
                    for j in range(4):
                        nc.tensor.matmul(
                            pt[:, :],
                            lhsT=xTu[:, j, sc * 128 : (sc + 1) * 128],
                            rhs=WoT_t[:, j, oc * 512 : (oc + 1) * 512],
                            start=(j == 0),
                            stop=(j == 3),
                        )
                    ot = outst.tile([128, 512], BF16, tag="out")
                    nc.vector.tensor_copy(out=ot[:, :], in_=pt[:, :])
                    nc.sync.dma_start(
                        out=out_d[sc * 128 : (sc + 1) * 128, oc * 512 : (oc + 1) * 512],
                        in_=ot[:, :],
                    )

            # ---- fused emission order (priority order for the scheduler)
            emit_qu(0)
            emit_ku(0)
            emit_vu(0)          # heads 0-3 (pairs j=0,1)
            chain(0, 0)
            emit_qu(1)
            emit_ku(1)
            chain(0, 1)
            emit_vu(1)          # heads 4-7 (pairs j=2,3)
            emit_qu(2)
            emit_ku(2)
            chain(0, 2)
            emit_qu(3)
            emit_ku(3)
            chain(0, 3)
            for sc in range(4):
                emit_ou(sc)
            for j in range(4):
                chain(1, j)
            for sc in range(4, 8):
                emit_ou(sc)


_NC = None


def _get_nc():
    global _NC
    if _NC is None:
        nc = bass.Bass(trn_type="TRN2")
        with tile.TileContext(nc) as tc:
            _emit(nc, tc)
        _legalize_waits(nc)
        _NC = nc
    return _NC


def _prep_inputs(xq, xkv, Wq, Wkv, Wout):
    import ml_dtypes

    bf = ml_dtypes.bfloat16
    xq = np.asarray(xq, dtype=np.float32)
    xkv = np.asarray(xkv, dtype=np.float32)
    Wq = np.asarray(Wq, dtype=np.float32)
    Wkv = np.asarray(Wkv, dtype=np.float32)
    Wout = np.asarray(Wout, dtype=np.float32)

    onesA = np.zeros((1, 128), bf)
    onesA[0, 0:64] = 1.0
    onesB = np.zeros((1, 128), bf)
    onesB[0, 64:128] = 1.0

    xqT = [np.ascontiguousarray(xq[b].T).astype(bf) for b in range(B)]
    xkvT = [np.ascontiguousarray(xkv[b].T).astype(bf) for b in range(B)]

    per_hg = []
    for hg in range(2):
        hs = slice(hg * HL, (hg + 1) * HL)
        WqTh = np.ascontiguousarray(Wq[hs].T).astype(bf)
        WkTh = np.ascontiguousarray(Wkv[hs].T).astype(bf)
        WvTh = np.ascontiguousarray(
            Wkv[HIDDEN + hg * HL : HIDDEN + (hg + 1) * HL].T
        ).astype(bf)
        WoTh = np.ascontiguousarray(Wout[:, hs].T).astype(bf)
        per_hg.append((WqTh, WkTh, WvTh, WoTh))

    in_maps = []
    for c in range(NCORES):
        b, hg = c // 2, c % 2
        WqTh, WkTh, WvTh, WoTh = per_hg[hg]
        in_maps.append(
            {
                "xqT": xqT[b],
                "xkvT": xkvT[b],
                "WqT": WqTh,
                "WkT": WkTh,
                "WvT": WvTh,
                "WoT": WoTh,
                "onesA": onesA,
                "onesB": onesB,
            }
        )
    return in_maps


def run_sharded(xq, xkv, Wq, Wkv, Wout, bout, trace=False, **kwargs):
    """Build+run the SPMD kernel; returns (full_output, BassKernelResults)."""
    nc = _get_nc()
    in_maps = _prep_inputs(xq, xkv, Wq, Wkv, Wout)
    res = run_bass_kernel_spmd(
        nc, in_maps, core_ids=list(range(NCORES)), trace=trace, **kwargs
    )
    bout = np.asarray(bout, dtype=np.float32)
    out = np.empty((B, SQ, HIDDEN), np.float32)
    for b in range(B):
        out[b] = res.results[2 * b]["out"].astype(np.float32) + res.results[
            2 * b + 1
        ]["out"].astype(np.float32)
    out += bout[None, None, :]
    return out, res


def kernel(xq, xkv, Wq, Wkv, Wout, bout):
    out, _ = run_sharded(xq, xkv, Wq, Wkv, Wout, bout)
    return out


# revision 25
# speedup vs baseline: 1.2600x; 1.0005x over previous
"""Cross-attention block (nn_CABlock) on 8 TRN2 NeuronCores.

Reference (per batch b):
    q  = xq @ Wq.T            -> [SQ, H]   split heads [SQ, 16, 64]
    kv = xkv @ Wkv.T          -> [SKV, 2H] split [SKV, 2, 16, 64]
    att = softmax(q k^T / sqrt(64))
    x   = att @ v  (merge heads)
    out = x @ Wout.T + bout

Sharding: 8 cores = 4 batches x 2 head-groups (8 heads each).  Each core
computes its batch's projections restricted to its 8 heads, attention for
those heads, and a partial out-projection (contraction over its 512
hd-columns of Wout).  The out projection is split into two head-pair
halves (j=0,1 / j=2,3) so the tail after the last attention chain is
tiny; the host sums the 4 partials per batch and adds bout.

v4 design (all-bf16 fused pipeline):
  - Everything bf16 on the wire and in SBUF (fp8 was tried and measured:
    softmax averaging does NOT damp relative error, every fp8 link costs
    2-5% L2 vs the 2e-2 max-rel gate).  PSUM accumulates fp32.
  - No phases: q/k/v projection units, attention chains (scores -> exp
    -> att@v), and out-projection halves are emitted interleaved.  Each
    chain carries a *filler* list of projection/out-proj units that are
    emitted two per y-block inside the chain, so the priority-heap Tile
    scheduler always has ready PE work while the scalar engine grinds
    the exp stream (chain ACT 17.7us > chain PE 14us).
  - PSUM (8 banks): 2 rotating score slots [128,2,512] (4 banks), po
    A/B att@v accumulators [65,512] (2 banks), 2 shared [128,512] slots
    for proj/bcast/out-proj (2 banks).
  - softmax denominator comes from a ones-column in the v tile (att @
    v_aug emits Z as row 64); normalization = ones-matmul partition
    broadcast + DVE reciprocal + one in-place multiply.
"""

import sys

sys.path.insert(0, "/opt/trn_rl_repo")

import numpy as np

import concourse.bass as bass
import concourse.mybir as mybir
import concourse.tile as tile
from concourse.bass_utils import run_bass_kernel_spmd

F32 = mybir.dt.float32
BF16 = mybir.dt.bfloat16

HIDDEN = 1024
NUM_HEADS = 16
HEAD_DIM = 64
B = 4
SQ = 1024
SKV = 2048
NCORES = 8
NHL = 8          # heads per core
HL = NHL * HEAD_DIM  # 512, local hd width
SCALE = HEAD_DIM ** -0.5
KCH = HIDDEN // 128  # 8 contraction chunks for the projections
NYC = SKV // 128     # 16 key/value row chunks


def _legalize_waits(nc, limit=1):
    """The walrus build in this container accepts only ~1 sync-wait per
    instruction struct; spill excess waits onto preceding engine NoOps."""
    import bass_rust

    ctr = 0
    for fn in nc.m.functions:
        for blk in fn.blocks:
            out = []
            changed = False
            for inst in blk.instructions:
                si = inst.sync_info
                ws = list(si.on_wait) if si is not None and si.on_wait else []
                if len(ws) > limit:
                    spill, keep = ws[:-limit], ws[-limit:]
                    for w in spill:
                        ctr += 1
                        nop = mybir.InstNoOp(name=f"ant-waitnop-{ctr}", ins=[], outs=[])
                        nop.engine = inst.engine
                        nop.sync_info = bass_rust.SyncInfo(on_wait=[w], on_update=[])
                        out.append(nop)
                    si.on_wait = keep
                    changed = True
                out.append(inst)
            if changed:
                blk.instructions = out
    return ctr


def _emit(nc, tc, debug=False):
    xqT = nc.dram_tensor("xqT", [HIDDEN, SQ], BF16, kind="ExternalInput")
    xkvT = nc.dram_tensor("xkvT", [HIDDEN, SKV], BF16, kind="ExternalInput")
    WqT = nc.dram_tensor("WqT", [HIDDEN, HL], BF16, kind="ExternalInput")
    WkT = nc.dram_tensor("WkT", [HIDDEN, HL], BF16, kind="ExternalInput")
    WvT = nc.dram_tensor("WvT", [HIDDEN, HL], BF16, kind="ExternalInput")
    WoT = nc.dram_tensor("WoT", [HL, HIDDEN], BF16, kind="ExternalInput")
    onesA = nc.dram_tensor("onesA", [1, 128], BF16, kind="ExternalInput")
    onesB = nc.dram_tensor("onesB", [1, 128], BF16, kind="ExternalInput")
    outA_d = nc.dram_tensor("outA", [SQ, HIDDEN], BF16, kind="ExternalOutput")
    outB_d = nc.dram_tensor("outB", [SQ, HIDDEN], BF16, kind="ExternalOutput")

    with tc.tile_pool(name="persist", bufs=1) as pp:
        # persistent SBUF (bytes/partition):
        xqT_t = pp.tile([128, KCH, SQ], BF16)      # 16 KB
        xkvT_t = pp.tile([128, KCH, SKV], BF16)    # 32 KB
        WqT_t = pp.tile([128, KCH, HL], BF16)      # 8 KB
        WkT_t = pp.tile([128, KCH, HL], BF16)      # 8 KB
        WvT_t = pp.tile([128, KCH, HL], BF16)      # 8 KB
        WoT_t = pp.tile([128, 4, HIDDEN], BF16)    # 8 KB
        qT = pp.tile([128, 4, SQ], BF16)           # 8 KB
        kT = pp.tile([128, 4, SKV], BF16)          # 16 KB
        va = pp.tile([128, NYC, NHL, 65], BF16)    # 16.3 KB
        xTu = pp.tile([128, 4, SQ], BF16)          # 8 KB
        onesA_t = pp.tile([1, 128], BF16)
        onesB_t = pp.tile([1, 128], BF16)

        # ---- input DMAs: q-side stream on sync, kv-side stream on scalar
        nc.sync.dma_start(out=onesA_t, in_=onesA[:, :])
        nc.sync.dma_start(out=onesB_t, in_=onesB[:, :])
        nc.sync.dma_start(out=WqT_t, in_=WqT.rearrange("(k p) n -> p k n", p=128))
        for k in range(KCH):
            nc.sync.dma_start(out=xqT_t[:, k, :], in_=xqT[k * 128 : (k + 1) * 128, :])
        nc.sync.dma_start(out=WoT_t, in_=WoT.rearrange("(j p) n -> p j n", p=128))
        nc.scalar.dma_start(out=WkT_t, in_=WkT.rearrange("(k p) n -> p k n", p=128))
        for k in range(KCH):
            nc.scalar.dma_start(
                out=xkvT_t[:, k, :], in_=xkvT[k * 128 : (k + 1) * 128, :]
            )
        nc.scalar.dma_start(out=WvT_t, in_=WvT.rearrange("(k p) n -> p k n", p=128))

        with (
            tc.tile_pool(name="psS", bufs=1, space="PSUM") as psS,
            tc.tile_pool(name="psPO", bufs=1, space="PSUM") as psPO,
            tc.tile_pool(name="psP", bufs=1, space="PSUM") as psP,
            tc.tile_pool(name="attp", bufs=1) as attp,
            tc.tile_pool(name="zrp", bufs=4) as zrp,
            tc.tile_pool(name="rzp", bufs=2) as rzp,
            tc.tile_pool(name="outst", bufs=3) as outst,
        ):
            nc.vector.memset(va[:, :, :, 64:65], 1.0)

            _pcnt = [0]

            def p_slot():
                _pcnt[0] += 1
                return psP.tile([128, 512], F32, tag=f"P{_pcnt[0] % 2}", name="pslot")

            def qu(m, s):
                # q projection for head pair m, x window s
                def emit():
                    pt = p_slot()
                    for k in range(KCH):
                        nc.tensor.matmul(
                            pt[:, :],
                            lhsT=WqT_t[:, k, m * 128 : (m + 1) * 128],
                            rhs=xqT_t[:, k, s * 512 : (s + 1) * 512],
                            start=(k == 0),
                            stop=(k == KCH - 1),
                        )
                    nc.vector.tensor_copy(
                        out=qT[:, m, s * 512 : (s + 1) * 512], in_=pt[:, :]
                    )
                return emit

            def ku(m, yg):
                # k projection for head pair m, y window yg
                def emit():
                    pt = p_slot()
                    for k in range(KCH):
                        nc.tensor.matmul(
                            pt[:, :],
                            lhsT=WkT_t[:, k, m * 128 : (m + 1) * 128],
                            rhs=xkvT_t[:, k, yg * 512 : (yg + 1) * 512],
                            start=(k == 0),
                            stop=(k == KCH - 1),
                        )
                    nc.vector.tensor_copy(
                        out=kT[:, m, yg * 512 : (yg + 1) * 512], in_=pt[:, :]
                    )
                return emit

            def vu(half, yc):
                # v projection for heads [4*half, 4*half+4), y chunk yc
                hlo = half * 4

                def emit():
                    pt = p_slot()
                    for k in range(KCH):
                        nc.tensor.matmul(
                            pt[:, 0:256],
                            lhsT=xkvT_t[:, k, yc * 128 : (yc + 1) * 128],
                            rhs=WvT_t[:, k, hlo * 64 : (hlo + 4) * 64],
                            start=(k == 0),
                            stop=(k == KCH - 1),
                        )
                    nc.vector.tensor_copy(
                        out=va[:, yc, hlo : hlo + 4, 0:64],
                        in_=pt[:, 0:256].rearrange("p (h d) -> p h d", h=4),
                    )
                return emit

            def ou(sc, half):
                # half out projection (head pairs 2*half, 2*half+1) for s rows
                # [sc*128, (sc+1)*128)
                od = outA_d if half == 0 else outB_d

                def emit():
                    for oc in range(HIDDEN // 512):
                        pt = p_slot()
                        for jj in range(2):
                            j = 2 * half + jj
                            nc.tensor.matmul(
                                pt[:, :],
                                lhsT=xTu[:, j, sc * 128 : (sc + 1) * 128],
                                rhs=WoT_t[:, j, oc * 512 : (oc + 1) * 512],
                                start=(jj == 0),
                                stop=(jj == 1),
                            )
                        ot = outst.tile([128, 512], BF16, tag="out")
                        nc.vector.tensor_copy(out=ot[:, :], in_=pt[:, :])
                        nc.sync.dma_start(
                            out=od[
                                sc * 128 : (sc + 1) * 128, oc * 512 : (oc + 1) * 512
                            ],
                            in_=ot[:, :],
                        )
                return emit

            NYB = NYC // 2  # 8 double-y blocks per chain
            LAG = 3         # attv trails scores/exp by this many blocks
            _scnt = [0]
            _acnt = [0]

            def chain(xc, j, filler=()):
                filler = list(filler)
                hA, hB = 2 * j, 2 * j + 1
                poA = psPO.tile([65, 512], F32, tag="POA", name="poA")
                poB = psPO.tile([65, 512], F32, tag="POB", name="poB")
                attsA, attsB = [], []

                def scores(h, yb):
                    pq = (h % 2) * 64
                    m = h // 2
                    _scnt[0] += 1
                    pscr = psS.tile(
                        [128, 2, 512], F32, tag=f"S{_scnt[0] % 2}", name="pscr"
                    )
                    for i in range(2):
                        yc = 2 * yb + i
                        nc.tensor.matmul(
                            pscr[:, i, :],
                            lhsT=kT[pq : pq + 64, m, yc * 128 : (yc + 1) * 128],
                            rhs=qT[pq : pq + 64, m, xc * 512 : (xc + 1) * 512],
                            start=True,
                            stop=True,
                        )
                    _acnt[0] += 1
                    at = attp.tile(
                        [128, 2, 512], BF16, tag=f"att{_acnt[0] % 10}", name="at"
                    )
                    nc.scalar.activation(
                        out=at[:, :, :].rearrange("p a b -> p (a b)"),
                        in_=pscr[:, :, :].rearrange("p a b -> p (a b)"),
                        func=mybir.ActivationFunctionType.Exp,
                        scale=SCALE,
                    )
                    return at

                def attv(h, po, at, yb):
                    for i in range(2):
                        yc = 2 * yb + i
                        nc.tensor.matmul(
                            po[:, :],
                            lhsT=va[:, yc, h, :],
                            rhs=at[:, i, :],
                            start=(yc == 0),
                            stop=(yc == NYC - 1),
                        )

                for yb in range(NYB):
                    # feed the scheduler ready filler work before this block
                    for _ in range(2):
                        if filler:
                            filler.pop(0)()
                    attsA.append(scores(hA, yb))
                    attsB.append(scores(hB, yb))
                    if yb >= LAG:
                        attv(hA, poA, attsA[yb - LAG], yb - LAG)
                        attv(hB, poB, attsB[yb - LAG], yb - LAG)
                for yb in range(NYB - LAG, NYB):
                    attv(hA, poA, attsA[yb], yb)
                    attv(hB, poB, attsB[yb], yb)
                for f in filler:
                    f()

                zrs = []
                for hi, po in ((0, poA), (1, poB)):
                    ps_off = hi * 64
                    nc.vector.tensor_copy(
                        out=xTu[ps_off : ps_off + 64, j, xc * 512 : (xc + 1) * 512],
                        in_=po[0:64, :],
                    )
                    zr = zrp.tile([1, 512], BF16, tag=f"zr{hi}", name="zr")
                    nc.vector.tensor_copy(out=zr[0:1, :], in_=po[64:65, :])
                    zrs.append(zr)
                # broadcast Z of both heads across partitions, reciprocal,
                # then normalize xTu in place
                pb = p_slot()
                nc.tensor.matmul(
                    pb[:, :], lhsT=onesA_t[0:1, :], rhs=zrs[0][0:1, :],
                    start=True, stop=False,
                )
                nc.tensor.matmul(
                    pb[:, :], lhsT=onesB_t[0:1, :], rhs=zrs[1][0:1, :],
                    start=False, stop=True,
                )
                rz = rzp.tile([128, 512], F32, tag="rz", name="rz")
                nc.vector.reciprocal(out=rz[:, :], in_=pb[:, :])
                nc.vector.tensor_mul(
                    xTu[:, j, xc * 512 : (xc + 1) * 512],
                    xTu[:, j, xc * 512 : (xc + 1) * 512],
                    rz[:, :],
                )

            # ---- fused emission order (priority order for the scheduler)
            qu(0, 0)()
            for yg in range(4):
                ku(0, yg)()
            qu(0, 1)()
            chain(0, 0, filler=[vu(0, yc) for yc in range(NYC)])
            qu(1, 0)()
            for yg in range(4):
                ku(1, yg)()
            qu(1, 1)()
            chain(0, 1, filler=[vu(1, yc) for yc in range(NYC)])
            qu(2, 0)()
            for yg in range(4):
                ku(2, yg)()
            qu(2, 1)()
            chain(0, 2, filler=[qu(3, 0), ku(3, 0), ku(3, 1), ku(3, 2),
                                ku(3, 3), qu(3, 1)])
            chain(0, 3, filler=[ou(0, 0), ou(1, 0), ou(2, 0), ou(3, 0)])
            chain(1, 0, filler=[ou(0, 1), ou(1, 1), ou(2, 1), ou(3, 1)])
            chain(1, 1)
            chain(1, 2, filler=[ou(4, 0), ou(5, 0), ou(6, 0), ou(7, 0)])
            chain(1, 3)
            for sc in range(4, 8):
                ou(sc, 1)()

            if debug:
                qT_d = nc.dram_tensor("qT_d", [128, 4, SQ], BF16, kind="ExternalOutput")
                kT_d = nc.dram_tensor("kT_d", [128, 4, SKV], BF16, kind="ExternalOutput")
                va_d = nc.dram_tensor("va_d", [128, NYC, NHL, 65], BF16, kind="ExternalOutput")
                xTu_d = nc.dram_tensor("xTu_d", [128, 4, SQ], BF16, kind="ExternalOutput")
                nc.sync.dma_start(out=qT_d[:, :, :], in_=qT[:, :, :])
                nc.sync.dma_start(out=kT_d[:, :, :], in_=kT[:, :, :])
                nc.sync.dma_start(out=va_d[:, :, :, :], in_=va[:, :, :, :])
                nc.sync.dma_start(out=xTu_d[:, :, :], in_=xTu[:, :, :])


_NC = None


def _get_nc():
    global _NC
    if _NC is None:
        nc = bass.Bass(trn_type="TRN2")
        with tile.TileContext(nc) as tc:
            _emit(nc, tc)
        _legalize_waits(nc)
        _NC = nc
    return _NC


def _prep_inputs(xq, xkv, Wq, Wkv, Wout):
    import ml_dtypes

    bf = ml_dtypes.bfloat16
    xq = np.asarray(xq, dtype=np.float32)
    xkv = np.asarray(xkv, dtype=np.float32)
    Wq = np.asarray(Wq, dtype=np.float32)
    Wkv = np.asarray(Wkv, dtype=np.float32)
    Wout = np.asarray(Wout, dtype=np.float32)

    onesA = np.zeros((1, 128), bf)
    onesA[0, 0:64] = 1.0
    onesB = np.zeros((1, 128), bf)
    onesB[0, 64:128] = 1.0

    xqT = [np.ascontiguousarray(xq[b].T).astype(bf) for b in range(B)]
    xkvT = [np.ascontiguousarray(xkv[b].T).astype(bf) for b in range(B)]

    per_hg = []
    for hg in range(2):
        hs = slice(hg * HL, (hg + 1) * HL)
        WqTh = np.ascontiguousarray(Wq[hs].T).astype(bf)
        WkTh = np.ascontiguousarray(Wkv[hs].T).astype(bf)
        WvTh = np.ascontiguousarray(
            Wkv[HIDDEN + hg * HL : HIDDEN + (hg + 1) * HL].T
        ).astype(bf)
        WoTh = np.ascontiguousarray(Wout[:, hs].T).astype(bf)
        per_hg.append((WqTh, WkTh, WvTh, WoTh))

    in_maps = []
    for c in range(NCORES):
        b, hg = c // 2, c % 2
        WqTh, WkTh, WvTh, WoTh = per_hg[hg]
        in_maps.append(
            {
                "xqT": xqT[b],
                "xkvT": xkvT[b],
                "WqT": WqTh,
                "WkT": WkTh,
                "WvT": WvTh,
                "WoT": WoTh,
                "onesA": onesA,
                "onesB": onesB,
            }
        )
    return in_maps


def run_sharded(xq, xkv, Wq, Wkv, Wout, bout, trace=False, **kwargs):
    """Build+run the SPMD kernel; returns (full_output, BassKernelResults)."""
    nc = _get_nc()
    in_maps = _prep_inputs(xq, xkv, Wq, Wkv, Wout)
    res = run_bass_kernel_spmd(
        nc, in_maps, core_ids=list(range(NCORES)), trace=trace, **kwargs
    )
    bout = np.asarray(bout, dtype=np.float32)
    out = np.empty((B, SQ, HIDDEN), np.float32)
    for b in range(B):
        out[b] = (
            res.results[2 * b]["outA"].astype(np.float32)
            + res.results[2 * b]["outB"].astype(np.float32)
            + res.results[2 * b + 1]["outA"].astype(np.float32)
            + res.results[2 * b + 1]["outB"].astype(np.float32)
        )
    out += bout[None, None, :]
    return out, res


def kernel(xq, xkv, Wq, Wkv, Wout, bout):
    out, _ = run_sharded(xq, xkv, Wq, Wkv, Wout, bout)
    return out


# revision 32
# speedup vs baseline: 1.3082x; 1.0382x over previous
"""Cross-attention block (nn_CABlock) on 8 TRN2 NeuronCores.

Reference (per batch b):
    q  = xq @ Wq.T            -> [SQ, H]   split heads [SQ, 16, 64]
    kv = xkv @ Wkv.T          -> [SKV, 2H] split [SKV, 2, 16, 64]
    att = softmax(q k^T / sqrt(64))
    x   = att @ v  (merge heads)
    out = x @ Wout.T + bout

Sharding: 8 cores = 4 batches x 2 head-groups (8 heads each).  Each core
computes its batch's projections restricted to its 8 heads, attention for
those heads, and a partial out-projection (contraction over its 512
hd-columns of Wout).  The out projection is split into two head-pair
halves (j=0,1 / j=2,3) so the tail after the last attention chain is
tiny; the host sums the 4 partials per batch and adds bout.

v4 design (all-bf16 fused pipeline):
  - Everything bf16 on the wire and in SBUF (fp8 was tried and measured:
    softmax averaging does NOT damp relative error, every fp8 link costs
    2-5% L2 vs the 2e-2 max-rel gate).  PSUM accumulates fp32.
  - No phases: q/k/v projection units, attention chains (scores -> exp
    -> att@v), and out-projection halves are emitted interleaved.  Each
    chain carries a *filler* list of projection/out-proj units that are
    emitted two per y-block inside the chain, so the priority-heap Tile
    scheduler always has ready PE work while the scalar engine grinds
    the exp stream (chain ACT 17.7us > chain PE 14us).
  - PSUM (8 banks): 2 rotating score slots [128,2,512] (4 banks), po
    A/B att@v accumulators [65,512] (2 banks), 2 shared [128,512] slots
    for proj/bcast/out-proj (2 banks).
  - softmax denominator comes from a ones-column in the v tile (att @
    v_aug emits Z as row 64); normalization = ones-matmul partition
    broadcast + DVE reciprocal + one in-place multiply.
"""

import sys

sys.path.insert(0, "/opt/trn_rl_repo")

import numpy as np

import concourse.bass as bass
import concourse.mybir as mybir
import concourse.tile as tile
from concourse.bass_utils import run_bass_kernel_spmd

F32 = mybir.dt.float32
BF16 = mybir.dt.bfloat16

HIDDEN = 1024
NUM_HEADS = 16
HEAD_DIM = 64
B = 4
SQ = 1024
SKV = 2048
NCORES = 8
NHL = 8          # heads per core
HL = NHL * HEAD_DIM  # 512, local hd width
SCALE = HEAD_DIM ** -0.5
KCH = HIDDEN // 128  # 8 contraction chunks for the projections
NYC = SKV // 128     # 16 key/value row chunks


def _legalize_waits(nc, limit=1):
    """The walrus build in this container accepts only ~1 sync-wait per
    instruction struct; spill excess waits onto preceding engine NoOps."""
    import bass_rust

    ctr = 0
    for fn in nc.m.functions:
        for blk in fn.blocks:
            out = []
            changed = False
            for inst in blk.instructions:
                si = inst.sync_info
                ws = list(si.on_wait) if si is not None and si.on_wait else []
                if len(ws) > limit:
                    spill, keep = ws[:-limit], ws[-limit:]
                    for w in spill:
                        ctr += 1
                        nop = mybir.InstNoOp(name=f"ant-waitnop-{ctr}", ins=[], outs=[])
                        nop.engine = inst.engine
                        nop.sync_info = bass_rust.SyncInfo(on_wait=[w], on_update=[])
                        out.append(nop)
                    si.on_wait = keep
                    changed = True
                out.append(inst)
            if changed:
                blk.instructions = out
    return ctr


def _emit(nc, tc, debug=False):
    xqT = nc.dram_tensor("xqT", [HIDDEN, SQ], BF16, kind="ExternalInput")
    xkvT = nc.dram_tensor("xkvT", [HIDDEN, SKV], BF16, kind="ExternalInput")
    WqT = nc.dram_tensor("WqT", [HIDDEN, HL], BF16, kind="ExternalInput")
    WkT = nc.dram_tensor("WkT", [HIDDEN, HL], BF16, kind="ExternalInput")
    WvT = nc.dram_tensor("WvT", [HIDDEN, HL], BF16, kind="ExternalInput")
    WoT = nc.dram_tensor("WoT", [HL, HIDDEN], BF16, kind="ExternalInput")
    onesA = nc.dram_tensor("onesA", [1, 128], BF16, kind="ExternalInput")
    onesB = nc.dram_tensor("onesB", [1, 128], BF16, kind="ExternalInput")
    # one partial per head pair j; host sums all of them (+ the other core's)
    out_ds = [
        nc.dram_tensor(f"out{j}", [SQ, HIDDEN], BF16, kind="ExternalOutput")
        for j in range(4)
    ]

    with tc.tile_pool(name="persist", bufs=1) as pp:
        # persistent SBUF (bytes/partition):
        xqT_t = pp.tile([128, KCH, SQ], BF16)      # 16 KB
        xkvT_t = pp.tile([128, KCH, SKV], BF16)    # 32 KB
        WqT_t = pp.tile([128, KCH, HL], BF16)      # 8 KB
        WkT_t = pp.tile([128, KCH, HL], BF16)      # 8 KB
        WvT_t = pp.tile([128, KCH, HL], BF16)      # 8 KB
        WoT_t = pp.tile([128, 4, HIDDEN], BF16)    # 8 KB
        qT = pp.tile([128, 4, SQ], BF16)           # 8 KB
        kT = pp.tile([128, 4, SKV], BF16)          # 16 KB
        va = pp.tile([128, NYC, NHL, 65], BF16)    # 16.3 KB
        xTu = pp.tile([128, 4, SQ], BF16)          # 8 KB
        onesA_t = pp.tile([1, 128], BF16)
        onesB_t = pp.tile([1, 128], BF16)

        # ---- input DMAs: q-side stream on sync, kv-side stream on scalar.
        # Wq/xq are chunk-interleaved so the k-th accumulation step of the
        # first q-proj unit unblocks as soon as its two chunks land.
        nc.sync.dma_start(out=onesA_t, in_=onesA[:, :])
        nc.sync.dma_start(out=onesB_t, in_=onesB[:, :])
        for k in range(KCH):
            nc.sync.dma_start(
                out=WqT_t[:, k, :], in_=WqT[k * 128 : (k + 1) * 128, :]
            )
            nc.sync.dma_start(out=xqT_t[:, k, :], in_=xqT[k * 128 : (k + 1) * 128, :])
        nc.sync.dma_start(out=WoT_t, in_=WoT.rearrange("(j p) n -> p j n", p=128))
        nc.scalar.dma_start(out=WkT_t, in_=WkT.rearrange("(k p) n -> p k n", p=128))
        for k in range(KCH):
            nc.scalar.dma_start(
                out=xkvT_t[:, k, :], in_=xkvT[k * 128 : (k + 1) * 128, :]
            )
        nc.scalar.dma_start(out=WvT_t, in_=WvT.rearrange("(k p) n -> p k n", p=128))

        with (
            tc.tile_pool(name="psS", bufs=1, space="PSUM") as psS,
            tc.tile_pool(name="psPO", bufs=1, space="PSUM") as psPO,
            tc.tile_pool(name="psP", bufs=1, space="PSUM") as psP,
            tc.tile_pool(name="attp", bufs=1) as attp,
            tc.tile_pool(name="zrp", bufs=4) as zrp,
            tc.tile_pool(name="rzp", bufs=2) as rzp,
            tc.tile_pool(name="outst", bufs=3) as outst,
        ):
            nc.vector.memset(va[:, :, :, 64:65], 1.0)

            _pcnt = [0]
            _wide = [True]  # before chain 0's att@v, the PO banks are free

            def p_slot():
                _pcnt[0] += 1
                if _wide[0]:
                    tag = ("P0", "P1", "POA", "POB")[_pcnt[0] % 4]
                    pool = psPO if tag.startswith("PO") else psP
                    return pool.tile([128, 512], F32, tag=tag, name="pslot")
                return psP.tile([128, 512], F32, tag=f"P{_pcnt[0] % 2}", name="pslot")

            def qu(m, s):
                # q projection for head pair m, x window s
                def emit():
                    pt = p_slot()
                    for k in range(KCH):
                        nc.tensor.matmul(
                            pt[:, :],
                            lhsT=WqT_t[:, k, m * 128 : (m + 1) * 128],
                            rhs=xqT_t[:, k, s * 512 : (s + 1) * 512],
                            start=(k == 0),
                            stop=(k == KCH - 1),
                        )
                    nc.vector.tensor_copy(
                        out=qT[:, m, s * 512 : (s + 1) * 512], in_=pt[:, :]
                    )
                return emit

            def ku(m, yg):
                # k projection for head pair m, y window yg
                def emit():
                    pt = p_slot()
                    for k in range(KCH):
                        nc.tensor.matmul(
                            pt[:, :],
                            lhsT=WkT_t[:, k, m * 128 : (m + 1) * 128],
                            rhs=xkvT_t[:, k, yg * 512 : (yg + 1) * 512],
                            start=(k == 0),
                            stop=(k == KCH - 1),
                        )
                    nc.vector.tensor_copy(
                        out=kT[:, m, yg * 512 : (yg + 1) * 512], in_=pt[:, :]
                    )
                return emit

            def vu(half, yc):
                # v projection for heads [4*half, 4*half+4), y chunk yc
                hlo = half * 4

                def emit():
                    pt = p_slot()
                    for k in range(KCH):
                        nc.tensor.matmul(
                            pt[:, 0:256],
                            lhsT=xkvT_t[:, k, yc * 128 : (yc + 1) * 128],
                            rhs=WvT_t[:, k, hlo * 64 : (hlo + 4) * 64],
                            start=(k == 0),
                            stop=(k == KCH - 1),
                        )
                    nc.vector.tensor_copy(
                        out=va[:, yc, hlo : hlo + 4, 0:64],
                        in_=pt[:, 0:256].rearrange("p (h d) -> p h d", h=4),
                    )
                return emit

            def ou(sc, j):
                # single-pair out projection partial for s rows
                # [sc*128, (sc+1)*128); gated only on chain (sc//4, j)
                od = out_ds[j]

                def emit():
                    for oc in range(HIDDEN // 512):
                        pt = p_slot()
                        nc.tensor.matmul(
                            pt[:, :],
                            lhsT=xTu[:, j, sc * 128 : (sc + 1) * 128],
                            rhs=WoT_t[:, j, oc * 512 : (oc + 1) * 512],
                            start=True,
                            stop=True,
                        )
                        ot = outst.tile([128, 512], BF16, tag="out")
                        nc.vector.tensor_copy(out=ot[:, :], in_=pt[:, :])
                        nc.sync.dma_start(
                            out=od[
                                sc * 128 : (sc + 1) * 128, oc * 512 : (oc + 1) * 512
                            ],
                            in_=ot[:, :],
                        )
                return emit

            NYB = NYC // 2  # 8 double-y blocks per chain
            LAG = 3         # attv trails scores/exp by this many blocks
            _scnt = [0]
            _acnt = [0]

            def chain(xc, j, filler=()):
                filler = list(filler)
                hA, hB = 2 * j, 2 * j + 1
                poA = psPO.tile([65, 512], F32, tag="POA", name="poA")
                poB = psPO.tile([65, 512], F32, tag="POB", name="poB")
                attsA, attsB = [], []

                def scores(h, yb):
                    pq = (h % 2) * 64
                    m = h // 2
                    _scnt[0] += 1
                    pscr = psS.tile(
                        [128, 2, 512], F32, tag=f"S{_scnt[0] % 2}", name="pscr"
                    )
                    for i in range(2):
                        yc = 2 * yb + i
                        nc.tensor.matmul(
                            pscr[:, i, :],
                            lhsT=kT[pq : pq + 64, m, yc * 128 : (yc + 1) * 128],
                            rhs=qT[pq : pq + 64, m, xc * 512 : (xc + 1) * 512],
                            start=True,
                            stop=True,
                        )
                    _acnt[0] += 1
                    at = attp.tile(
                        [128, 2, 512], BF16, tag=f"att{_acnt[0] % 10}", name="at"
                    )
                    nc.scalar.activation(
                        out=at[:, :, :].rearrange("p a b -> p (a b)"),
                        in_=pscr[:, :, :].rearrange("p a b -> p (a b)"),
                        func=mybir.ActivationFunctionType.Exp,
                        scale=SCALE,
                    )
                    return at

                def attv(h, po, at, yb):
                    for i in range(2):
                        yc = 2 * yb + i
                        nc.tensor.matmul(
                            po[:, :],
                            lhsT=va[:, yc, h, :],
                            rhs=at[:, i, :],
                            start=(yc == 0),
                            stop=(yc == NYC - 1),
                        )

                for yb in range(NYB):
                    # feed the scheduler ready filler work before this block
                    for _ in range(2):
                        if filler:
                            filler.pop(0)()
                    attsA.append(scores(hA, yb))
                    attsB.append(scores(hB, yb))
                    if yb >= LAG:
                        attv(hA, poA, attsA[yb - LAG], yb - LAG)
                        attv(hB, poB, attsB[yb - LAG], yb - LAG)
                for yb in range(NYB - LAG, NYB):
                    attv(hA, poA, attsA[yb], yb)
                    attv(hB, poB, attsB[yb], yb)
                for f in filler:
                    f()

                zrs = []
                for hi, po in ((0, poA), (1, poB)):
                    ps_off = hi * 64
                    nc.vector.tensor_copy(
                        out=xTu[ps_off : ps_off + 64, j, xc * 512 : (xc + 1) * 512],
                        in_=po[0:64, :],
                    )
                    zr = zrp.tile([1, 512], BF16, tag=f"zr{hi}", name="zr")
                    nc.vector.tensor_copy(out=zr[0:1, :], in_=po[64:65, :])
                    zrs.append(zr)
                # broadcast Z of both heads across partitions, reciprocal,
                # then normalize xTu in place.  pb lives in the POA bank: its
                # deps already chain through po, and this keeps the P slots
                # free for projection filler at chain boundaries.
                pb = psPO.tile([128, 512], F32, tag="POA", name="pb")
                nc.tensor.matmul(
                    pb[:, :], lhsT=onesA_t[0:1, :], rhs=zrs[0][0:1, :],
                    start=True, stop=False,
                )
                nc.tensor.matmul(
                    pb[:, :], lhsT=onesB_t[0:1, :], rhs=zrs[1][0:1, :],
                    start=False, stop=True,
                )
                rz = rzp.tile([128, 512], F32, tag="rz", name="rz")
                nc.vector.reciprocal(out=rz[:, :], in_=pb[:, :])
                nc.vector.tensor_mul(
                    xTu[:, j, xc * 512 : (xc + 1) * 512],
                    xTu[:, j, xc * 512 : (xc + 1) * 512],
                    rz[:, :],
                )

            # ---- fused emission order (priority order for the scheduler)
            qu(0, 0)()
            for yg in range(4):
                ku(0, yg)()
            qu(0, 1)()
            _wide[0] = False  # PO banks belong to the attention chains now
            chain(0, 0, filler=[vu(0, yc) for yc in range(NYC)])
            qu(1, 0)()
            for yg in range(4):
                ku(1, yg)()
            qu(1, 1)()
            chain(0, 1, filler=[vu(1, yc) for yc in range(NYC)])
            qu(2, 0)()
            for yg in range(4):
                ku(2, yg)()
            qu(2, 1)()
            chain(0, 2, filler=[qu(3, 0), ku(3, 0), ku(3, 1), ku(3, 2),
                                ku(3, 3), qu(3, 1),
                                ou(0, 0), ou(1, 0), ou(2, 0), ou(3, 0)])
            chain(0, 3, filler=[ou(0, 1), ou(1, 1), ou(2, 1), ou(3, 1)])
            chain(1, 0, filler=[ou(0, 2), ou(1, 2), ou(2, 2), ou(3, 2)])
            chain(1, 1, filler=[ou(0, 3), ou(1, 3), ou(2, 3), ou(3, 3),
                                ou(4, 0), ou(5, 0), ou(6, 0), ou(7, 0)])
            chain(1, 2, filler=[ou(4, 1), ou(5, 1), ou(6, 1), ou(7, 1)])
            chain(1, 3, filler=[ou(4, 2), ou(5, 2), ou(6, 2), ou(7, 2)])
            for sc in range(4, 8):
                ou(sc, 3)()

            if debug:
                qT_d = nc.dram_tensor("qT_d", [128, 4, SQ], BF16, kind="ExternalOutput")
                kT_d = nc.dram_tensor("kT_d", [128, 4, SKV], BF16, kind="ExternalOutput")
                va_d = nc.dram_tensor("va_d", [128, NYC, NHL, 65], BF16, kind="ExternalOutput")
                xTu_d = nc.dram_tensor("xTu_d", [128, 4, SQ], BF16, kind="ExternalOutput")
                nc.sync.dma_start(out=qT_d[:, :, :], in_=qT[:, :, :])
                nc.sync.dma_start(out=kT_d[:, :, :], in_=kT[:, :, :])
                nc.sync.dma_start(out=va_d[:, :, :, :], in_=va[:, :, :, :])
                nc.sync.dma_start(out=xTu_d[:, :, :], in_=xTu[:, :, :])


_NC = None


def _get_nc():
    global _NC
    if _NC is None:
        nc = bass.Bass(trn_type="TRN2")
        with tile.TileContext(nc) as tc:
            _emit(nc, tc)
        _legalize_waits(nc)
        _NC = nc
    return _NC


def _prep_inputs(xq, xkv, Wq, Wkv, Wout):
    import ml_dtypes

    bf = ml_dtypes.bfloat16
    xq = np.asarray(xq, dtype=np.float32)
    xkv = np.asarray(xkv, dtype=np.float32)
    Wq = np.asarray(Wq, dtype=np.float32)
    Wkv = np.asarray(Wkv, dtype=np.float32)
    Wout = np.asarray(Wout, dtype=np.float32)

    onesA = np.zeros((1, 128), bf)
    onesA[0, 0:64] = 1.0
    onesB = np.zeros((1, 128), bf)
    onesB[0, 64:128] = 1.0

    xqT = [np.ascontiguousarray(xq[b].T).astype(bf) for b in range(B)]
    xkvT = [np.ascontiguousarray(xkv[b].T).astype(bf) for b in range(B)]

    per_hg = []
    for hg in range(2):
        hs = slice(hg * HL, (hg + 1) * HL)
        WqTh = np.ascontiguousarray(Wq[hs].T).astype(bf)
        WkTh = np.ascontiguousarray(Wkv[hs].T).astype(bf)
        WvTh = np.ascontiguousarray(
            Wkv[HIDDEN + hg * HL : HIDDEN + (hg + 1) * HL].T
        ).astype(bf)
        WoTh = np.ascontiguousarray(Wout[:, hs].T).astype(bf)
        per_hg.append((WqTh, WkTh, WvTh, WoTh))

    in_maps = []
    for c in range(NCORES):
        b, hg = c // 2, c % 2
        WqTh, WkTh, WvTh, WoTh = per_hg[hg]
        in_maps.append(
            {
                "xqT": xqT[b],
                "xkvT": xkvT[b],
                "WqT": WqTh,
                "WkT": WkTh,
                "WvT": WvTh,
                "WoT": WoTh,
                "onesA": onesA,
                "onesB": onesB,
            }
        )
    return in_maps


def run_sharded(xq, xkv, Wq, Wkv, Wout, bout, trace=False, **kwargs):
    """Build+run the SPMD kernel; returns (full_output, BassKernelResults)."""
    nc = _get_nc()
    in_maps = _prep_inputs(xq, xkv, Wq, Wkv, Wout)
    res = run_bass_kernel_spmd(
        nc, in_maps, core_ids=list(range(NCORES)), trace=trace, **kwargs
    )
    bout = np.asarray(bout, dtype=np.float32)
    out = np.empty((B, SQ, HIDDEN), np.float32)
    for b in range(B):
        acc = np.zeros((SQ, HIDDEN), np.float32)
        for c in (2 * b, 2 * b + 1):
            for j in range(4):
                acc += res.results[c][f"out{j}"].astype(np.float32)
        out[b] = acc
    out += bout[None, None, :]
    return out, res


def kernel(xq, xkv, Wq, Wkv, Wout, bout):
    out, _ = run_sharded(xq, xkv, Wq, Wkv, Wout, bout)
    return out
